# revision 19
# baseline (speedup 1.0000x reference)
"""Adaptive mean thresholding (11x11 box mean, replicate border, C=2, INV)
on 8 trn2 NeuronCores. Batch data-parallel: 16 images of [512,512] per core.

Algorithm per core:
  The separable 11x11 box *sum* S = F @ I @ F^T with F the 512x512 banded
  (integer-count) filter matrix that encodes replicate-border clamping.
  Both 1D passes run on TensorE with the *data chunk as lhsT* (stationary)
  and constant F^T band-windows as rhs (moving), so each pass transposes
  the data; two passes land back in the original orientation with zero
  explicit transposes:
     pass1: VT[w, h'] = sum_h I[h, w] * FT[h, h']      (lhsT = I chunks)
     pass2: U[h', w'] = sum_w VT[w, h'] * FT[w, w']    (lhsT = VT chunks)
  Matmuls run in fp16 (10-bit mantissa, same as tf32, but fast weight
  load) with fp32 PSUM accumulation; inputs are mean-centered (free bias
  folds on existing ACT copies) so fp16 quantization error stays ~2e-3
  relative on the box mean. The threshold compare is
     out = 255 * (S >= 121*I + 242)   (exactly: 0 if I > S/121 - 2 else 255)
  done as one DVE tensor_scalar pass (t2 = 121*I + bias), one DVE
  tensor_tensor is_ge directly against PSUM, one ACT *255 scale. Loads go
  out on the SP HWDGE ring, stores on the SWDGE (gpsimd) ring, compute
  engines (PE/ACT/DVE, each <60% of the DMA span) pipeline underneath;
  the kernel is at the per-core HBM roofline (~95 us for 33.6 MB).
"""

import sys

for p in ("/opt/trn_rl_repo", "/opt/trn_rl_repo/concourse"):
    if p not in sys.path:
        sys.path.insert(0, p)

import numpy as np

import concourse.bass as bass
import concourse.bacc as bacc
import concourse.mybir as mybir
import concourse.tile as tile
from concourse.bass_utils import run_bass_kernel_spmd

F32 = mybir.dt.float32
F16 = mybir.dt.float16

N_CORES = 8
B_PER_CORE = 16
H = W = 512
K = 11
PAD = K // 2
CONSTANT = 2.0
MAXVAL = 255.0
# which engine issues output DMAs ("scalar" -> qActDynamicHW ring,
# "gpsimd" -> SWDGE, "sync" -> qSPDynamicHW shared with loads)
OUT_DMA_ENGINE = "gpsimd"
# DMA spread: cycles of engines per image index for loads/stores; None falls
# back to sync-loads / OUT_DMA_ENGINE-stores. More distinct rings engaged =>
# larger share of the SDMA packet round-robin when the chip is contended.
LOAD_ENGINES = None
STORE_ENGINES = None
SPLIT_DMA = False
# tail structure: "psum_dve" = DVE is_ge reads U from PSUM directly;
# "usb_act" = ACT evacuates U to SBUF first (DVE reads SBUF at 1x-fast init)
TAIL_MODE = "psum_dve"
# engine for the t2 = 121*I + bias pass
T2_ENGINE = "vector"
# engine for the final *255 scale
SCALE_ENGINE = "scalar"

# rhs/psum windows per 128-block of the contraction dim; window k must
# contain the band [128k-5, 128k+133). Window 0 spans the full bank so the
# single start=True matmul initializes every element (PSUM has_written is
# cleared bank-wide by start=True); windows 1-3 accumulate, 256 wide to
# keep float32r at full rate (needs output free dim >= 256).
# default: 256-wide accumulate windows (f32r-era layout, also fine for fp16)
# banded: exact band windows [128k-5, 128k+133) - fp16 has no N>=256 rule
BANDED = True
IMG_BUFS = 5
OUT_BUFS = 5


def _window_layout():
    if BANDED:
        win = (0, 123, 251, 379)
        widths = (512, 138, 138, 133)
    else:
        win = (0, 64, 192, 256)
        widths = (512, 256, 256, 256)
    off = [0]
    for w in widths[:-1]:
        off.append(off[-1] + w)
    return win, widths, tuple(off), off[-1] + widths[-1]


WIN, WIDTHS, FTW_OFF, FTW_TOTAL = _window_layout()

# V = vertical 11-sum of U[0,255] pixels; center to cut tf32 quant error.
VCENTER = 11 * 127.5  # 1402.5
# out = 255 iff S >= 121*I + 242. With pass2 input centered:
#   U' = S - 11*VCENTER  =>  S >= 121*I + 242  <=>  U' >= 121*I + 242 - 11*VCENTER
T2_BIAS = 242.0 - 11 * VCENTER  # -15185.5


def _filter_matrix() -> np.ndarray:
    """F[o, i] = number of taps of output o's clamped window hitting input i."""
    F = np.zeros((H, H), dtype=np.float64)
    for o in range(H):
        for d in range(-PAD, PAD + 1):
            F[o, min(max(o + d, 0), H - 1)] += 1.0
    return F


def _ftw_windows() -> np.ndarray:
    """[128, FTW_TOTAL]: FT[128k:128(k+1), WIN[k]:WIN[k]+WIDTHS[k]], concat."""
    WIN, WIDTHS, FTW_OFF, FTW_TOTAL = _window_layout()
    FT = _filter_matrix().T
    tiles = [
        FT[128 * k : 128 * (k + 1), WIN[k] : WIN[k] + WIDTHS[k]] for k in range(4)
    ]
    return np.ascontiguousarray(np.concatenate(tiles, axis=1)).astype(np.float16)


class _nullcontext:
    def __enter__(self):
        return None

    def __exit__(self, *a):
        return False


def _emit_images(nc, tc, pools, img_d, out_d, ftw):
    WIN, WIDTHS, FTW_OFF, FTW_TOTAL = _window_layout()
    (img_pool, imgr_pool, vt_pool, t2_pool, usb_pool, c01_pool, out_pool,
     vtps_pool, ups_pool) = pools
    load_cycle = LOAD_ENGINES or ("sync",)
    store_cycle = STORE_ENGINES or (OUT_DMA_ENGINE,)
    for b in range(B_PER_CORE):
        img = img_pool.tile([128, 4 * W], F32)
        src = img_d[b * H : (b + 1) * H, :].rearrange("(t p) w -> p t w", p=128)
        ld = getattr(nc, load_cycle[b % len(load_cycle)])
        if SPLIT_DMA:
            ld2 = getattr(nc, load_cycle[(b + 1) % len(load_cycle)])
            dstap = img[:].rearrange("p (t w) -> p t w", t=4)
            ld.dma_start(dstap[:, 0:2, :], src[:, 0:2, :])
            ld2.dma_start(dstap[:, 2:4, :], src[:, 2:4, :])
        else:
            ld.dma_start(img[:].rearrange("p (t w) -> p t w", t=4), src)

        # fp16 copy of the image for the pass-1 matmuls, centered so the
        # fp16 quantization error is minimal; the filter weights sum to 11
        # per output, so pass-1 output is exactly V - 11*127.5 = V - VCENTER
        # and the evacuation needs no further centering. The threshold
        # compare keeps reading the exact fp32 image.
        img_r = imgr_pool.tile([128, 4 * W], F16)
        nc.scalar.activation(
            img_r[:], img[:], mybir.ActivationFunctionType.Copy, bias=-127.5
        )

        # pass 1: VT[wblk j] <- sum over row-blocks k of I-chunk^T @ FTwin
        vt_ps = vtps_pool.tile([128, 4 * W], F32)
        for j in range(4):
            for k in range(4):
                nc.tensor.matmul(
                    vt_ps[:, j * 512 + WIN[k] : j * 512 + WIN[k] + WIDTHS[k]],
                    img_r[:, k * 512 + j * 128 : k * 512 + j * 128 + 128],
                    ftw[:, FTW_OFF[k] : FTW_OFF[k] + WIDTHS[k]],
                    start=(k == 0),
                    stop=(k == 3),
                )

        # evacuate PSUM (already centered by the img_r bias)
        vt_sb = vt_pool.tile([128, 4 * W], F16)
        nc.scalar.activation(
            vt_sb[:], vt_ps[:], mybir.ActivationFunctionType.Copy
        )

        # pass 2: U[hblk t] <- sum over col-blocks k of VT-chunk^T @ FTwin
        u_ps = ups_pool.tile([128, 4 * W], F32)
        for t in range(4):
            for k in range(4):
                nc.tensor.matmul(
                    u_ps[:, t * 512 + WIN[k] : t * 512 + WIN[k] + WIDTHS[k]],
                    vt_sb[:, k * 512 + t * 128 : k * 512 + t * 128 + 128],
                    ftw[:, FTW_OFF[k] : FTW_OFF[k] + WIDTHS[k]],
                    start=(k == 0),
                    stop=(k == 3),
                )

        # threshold: out = 255 * (U' >= 121*I + T2_BIAS)
        t2 = t2_pool.tile([128, 4 * W], F32)
        if T2_ENGINE == "scalar":
            nc.scalar.activation(
                t2[:],
                img[:],
                mybir.ActivationFunctionType.Copy,
                bias=T2_BIAS,
                scale=121.0,
            )
        else:
            nc.vector.tensor_scalar(
                t2[:], img[:], 121.0, T2_BIAS,
                mybir.AluOpType.mult, mybir.AluOpType.add,
            )
        if TAIL_MODE == "usb_act":
            u_sb = usb_pool.tile([128, 4 * W], F32)
            nc.scalar.activation(
                u_sb[:], u_ps[:], mybir.ActivationFunctionType.Copy
            )
            u_src = u_sb
        else:
            u_src = u_ps
        c01 = c01_pool.tile([128, 4 * W], F32)
        nc.vector.tensor_tensor(c01[:], u_src[:], t2[:], mybir.AluOpType.is_ge)
        outt = out_pool.tile([128, 4 * W], F32)
        if SCALE_ENGINE == "vector":
            nc.vector.tensor_scalar_mul(outt[:], c01[:], MAXVAL)
        else:
            nc.scalar.activation(
                outt[:], c01[:], mybir.ActivationFunctionType.Copy, scale=MAXVAL
            )

        dst = out_d[b * H : (b + 1) * H, :].rearrange("(t p) w -> p t w", p=128)
        st = getattr(nc, store_cycle[b % len(store_cycle)])
        if SPLIT_DMA:
            st2 = getattr(nc, store_cycle[(b + 1) % len(store_cycle)])
            outap = outt[:].rearrange("p (t w) -> p t w", t=4)
            st.dma_start(dst[:, 0:2, :], outap[:, 0:2, :])
            st2.dma_start(dst[:, 2:4, :], outap[:, 2:4, :])
        else:
            st.dma_start(dst, outt[:].rearrange("p (t w) -> p t w", t=4))


def _build_nc(reps: int = 1) -> bass.Bass:
    global WIN, WIDTHS, FTW_OFF, FTW_TOTAL
    WIN, WIDTHS, FTW_OFF, FTW_TOTAL = _window_layout()
    nc = bacc.Bacc()
    img_d = nc.declare_dram_parameter(
        "image", [B_PER_CORE * H, W], F32, isOutput=False
    )
    ftw_d = nc.declare_dram_parameter(
        "ftw", [128, FTW_TOTAL], F16, isOutput=False
    )
    out_d = nc.declare_dram_parameter("out", [B_PER_CORE * H, W], F32, isOutput=True)

    with tile.TileContext(nc) as tc:
        with (
            tc.tile_pool(name="const", bufs=1) as const_pool,
            tc.tile_pool(name="img", bufs=IMG_BUFS) as img_pool,
            tc.tile_pool(name="imgr", bufs=2) as imgr_pool,
            tc.tile_pool(name="vt", bufs=2) as vt_pool,
            tc.tile_pool(name="t2", bufs=2) as t2_pool,
            tc.tile_pool(name="usb", bufs=2) as usb_pool,
            tc.tile_pool(name="c01", bufs=2) as c01_pool,
            tc.tile_pool(name="outp", bufs=OUT_BUFS) as out_pool,
            tc.tile_pool(name="vtps", bufs=1, space="PSUM") as vtps_pool,
            tc.tile_pool(name="ups", bufs=1, space="PSUM") as ups_pool,
        ):
            pools = (img_pool, imgr_pool, vt_pool, t2_pool, usb_pool,
                     c01_pool, out_pool, vtps_pool, ups_pool)
            ftw = const_pool.tile([128, FTW_TOTAL], F16)
            nc.sync.dma_start(ftw[:], ftw_d[:])

            if reps > 1:
                # benchmark mode: run the whole pipeline reps times inside
                # the NEFF so per-call dispatch overhead amortizes away
                loop_ctx = tc.For_i(0, reps, 1)
            else:
                loop_ctx = _nullcontext()
            with loop_ctx:
                _emit_images(nc, tc, pools, img_d, out_d, ftw)

    nc.compile()
    return nc


_NC_CACHE = None


def _get_nc() -> bass.Bass:
    global _NC_CACHE
    if _NC_CACHE is None:
        _NC_CACHE = _build_nc()
    return _NC_CACHE


def kernel(image: np.ndarray) -> np.ndarray:
    assert image.shape == (128, H, W, 1), image.shape
    img = np.ascontiguousarray(image.reshape(128, H, W).astype(np.float32))
    ftw = _ftw_windows()

    in_maps = []
    for c in range(N_CORES):
        shard = img[c * B_PER_CORE : (c + 1) * B_PER_CORE].reshape(
            B_PER_CORE * H, W
        )
        in_maps.append({"image": np.ascontiguousarray(shard), "ftw": ftw})

    nc = _get_nc()
    res = run_bass_kernel_spmd(nc, in_maps, core_ids=list(range(N_CORES)))
    shards = [
        res.results[c]["out"].reshape(B_PER_CORE, H, W, 1) for c in range(N_CORES)
    ]
    return np.concatenate(shards, axis=0).astype(np.float32)
